# revision 6
# baseline (speedup 1.0000x reference)
"""Kernel attention (linear attention w/ elu+1 feature map) on 8 trn2 NeuronCores.

Problem: B=8, H=8, N=1024, D=64.
  phi(x) = elu(x) + 1
  S  = phi(Q) @ phi(K)^T          [B,H,N,N]
  P  = S @ V                      [B,H,N,N]  (dv == N)
  out = P / S                     elementwise

Sharding: batch b -> core b (8 heads per core, fully independent).

Per-core dataflow (per head):
  - load Q,K [1024,64], compute phi on-chip (fp32)
  - PE-transpose (2 heads packed per 128x128 transpose) -> phiQT/phiKT [64,1024]
  - ST[m,n] = phiK @ phiQ^T via f32r matmuls (lhsT=phiKT chunk), PSUM->SBUF cast to bf16
  - V loaded fp32, cast to bf16
  - per n-chunk (128 rows):
      S chunk via f32r matmul (lhsT=phiQT chunk)
      recipS = exp(-ln(S)) on ACT (ln/exp share one LUT table set)
      P chunk = sum_m ST^T-chunk @ V-chunk (bf16 matmuls, fp32 PSUM accum)
      out = P * recipS on DVE, DMA out
"""

import numpy as np
from contextlib import ExitStack

import concourse.bass as bass
import concourse.tile as tile
import concourse.mybir as mybir
from concourse import bacc
from concourse.bass_utils import run_bass_kernel_spmd
from concourse.masks import make_identity

P = 128
N_CORES = 8
HPC = 8          # heads per core (= H; batch is the sharded dim)
N = 1024
D = 64
NT = N // P      # 8
F32 = mybir.dt.float32
F32R = mybir.dt.float32r
BF16 = mybir.dt.bfloat16
AF = mybir.ActivationFunctionType
ALU = mybir.AluOpType

_cache = {}


def _patch_act_tables():
    """Force Exp and Ln to resolve to the single table set containing both
    (natural_log_exp_and_others), so the ACT LUT is loaded once instead of
    thrashing ~2.7us per Ln<->Exp alternation. Keys/order preserved, so
    act_func_set_id indices stay valid."""
    if _cache.get("tables_patched"):
        return
    orig = bacc.get_activation_tables

    def patched(arch):
        tabs = dict(orig(arch))
        both = [k for k, v in tabs.items() if AF.Exp in v and AF.Ln in v]
        if both:
            keep = both[0]
            tabs = {
                k: (v if k == keep else (set(v) - {AF.Exp, AF.Ln}))
                for k, v in tabs.items()
            }
        return tabs

    bacc.get_activation_tables = patched
    _cache["tables_patched"] = True


def _build():
    _patch_act_tables()
    nc = bacc.Bacc("TRN2", target_bir_lowering=False, debug=False, num_devices=N_CORES)
    Q = nc.dram_tensor("q", [HPC, N, D], F32, kind="ExternalInput").ap()
    K = nc.dram_tensor("k", [HPC, N, D], F32, kind="ExternalInput").ap()
    V = nc.dram_tensor("v", [HPC, N, N], F32, kind="ExternalInput").ap()
    O = nc.dram_tensor("o", [HPC, N, N], F32, kind="ExternalOutput").ap()

    Qr = Q.rearrange("h (t p) d -> h p t d", p=P)   # [8, 128, 8, 64]
    Kr = K.rearrange("h (t p) d -> h p t d", p=P)
    Vr = V.rearrange("h (m p) v -> h p m v", p=P)   # [8, 128, 8, 1024]

    with tile.TileContext(nc) as tc, ExitStack() as ctx:
        const = ctx.enter_context(tc.tile_pool(name="const", bufs=1))
        prep = ctx.enter_context(tc.tile_pool(name="prep", bufs=2))
        qkt = ctx.enter_context(tc.tile_pool(name="qkt", bufs=2))
        stp = ctx.enter_context(tc.tile_pool(name="stp", bufs=2))
        vp = ctx.enter_context(tc.tile_pool(name="vp", bufs=2))
        vstage = ctx.enter_context(tc.tile_pool(name="vstage", bufs=4))
        outp = ctx.enter_context(tc.tile_pool(name="outp", bufs=3))
        recp = ctx.enter_context(tc.tile_pool(name="recp", bufs=2))
        tps = ctx.enter_context(tc.tile_pool(name="tpsum", bufs=2, space="PSUM"))
        sps = ctx.enter_context(tc.tile_pool(name="spsum", bufs=2, space="PSUM"))
        pps = ctx.enter_context(tc.tile_pool(name="ppsum", bufs=2, space="PSUM"))

        ident = const.tile([P, P], F32)
        make_identity(nc, ident)

        for pair in range(HPC // 2):
            h0 = 2 * pair
            h1 = 2 * pair + 1
            # ---- phase A: load Q,K both heads, phi, transpose (2 heads packed)
            qT = [qkt.tile([D, N], F32R, tag=f"qT{i}", name=f"qT{i}") for i in range(2)]
            kT = [qkt.tile([D, N], F32R, tag=f"kT{i}", name=f"kT{i}") for i in range(2)]
            for raw_tag, src, dstT in (("qraw", Qr, qT), ("kraw", Kr, kT)):
                raw = prep.tile([P, NT, 2 * D], F32, tag=raw_tag, name=raw_tag)
                nc.sync.dma_start(raw[:, :, 0:D], src[h0])
                nc.sync.dma_start(raw[:, :, D:2 * D], src[h1])
                flat = raw.rearrange("p t d -> p (t d)")
                tmp = prep.tile([P, NT * 2 * D], F32, tag="tmp")
                # phi(x) = max(x+1, exp(min(x, 0)))
                nc.vector.tensor_scalar_min(tmp[:], flat, 0.0)
                nc.scalar.activation(tmp[:], tmp[:], AF.Exp)
                nc.vector.scalar_tensor_tensor(
                    flat, flat, 1.0, tmp[:], ALU.add, ALU.max
                )
                for t in range(NT):
                    ps = tps.tile([P, P], F32)
                    nc.tensor.transpose(ps[:], raw[:, t, :], ident[:])
                    nc.scalar.copy(dstT[0][:, t * P:(t + 1) * P], ps[0:D, :])
                    nc.vector.tensor_copy(dstT[1][:, t * P:(t + 1) * P], ps[D:2 * D, :])

            for hi, h in enumerate((h0, h1)):
                qTh = qT[hi]
                kTh = kT[hi]
                # ---- phase B: ST = phiK @ phiQ^T (m on partitions), cast bf16
                st = stp.tile([P, NT, N], BF16, tag="st")
                for m in range(NT):
                    s_ps = sps.tile([P, N], F32, tag="sps")
                    for half in range(2):
                        nc.tensor.matmul(
                            s_ps[:, half * 512:(half + 1) * 512],
                            kTh[:, m * P:(m + 1) * P],
                            qTh[:, half * 512:(half + 1) * 512],
                            start=True, stop=True,
                        )
                    nc.vector.tensor_copy(st[:, m, :], s_ps[:])
                # ---- V load with fp32->bf16 cast during DMA (SWDGE)
                vt = vp.tile([P, NT, N], BF16, tag="vt")
                for m in range(NT):
                    nc.gpsimd.dma_start(vt[:, m, :], Vr[h, :, m, :])
                # ---- phase C: per n-chunk
                for n in range(NT):
                    s_ps = sps.tile([P, N], F32, tag="sps")
                    for half in range(2):
                        nc.tensor.matmul(
                            s_ps[:, half * 512:(half + 1) * 512],
                            qTh[:, n * P:(n + 1) * P],
                            kTh[:, half * 512:(half + 1) * 512],
                            start=True, stop=True,
                        )
                    lnt = recp.tile([P, N], F32, tag="ln")
                    rec = recp.tile([P, N], F32, tag="rec")
                    nc.scalar.activation(lnt[:], s_ps[:], AF.Ln)
                    nc.scalar.activation(rec[:], lnt[:], AF.Exp, scale=-1.0)
                    outt = outp.tile([P, N], F32, tag="outt")
                    for v in range(2):
                        p_ps = pps.tile([P, 512], F32, tag="pp")
                        for m in range(NT):
                            nc.tensor.matmul(
                                p_ps[:],
                                st[:, m, n * P:(n + 1) * P],
                                vt[:, m, v * 512:(v + 1) * 512],
                                start=(m == 0), stop=(m == NT - 1),
                            )
                        nc.vector.tensor_mul(
                            outt[:, v * 512:(v + 1) * 512],
                            p_ps[:],
                            rec[:, v * 512:(v + 1) * 512],
                        )
                    nc.sync.dma_start(O[h, n * P:(n + 1) * P, :], outt[:])
    nc.compile()
    return nc


def _get_nc():
    if "nc" not in _cache:
        _cache["nc"] = _build()
    return _cache["nc"]


def kernel(Q, K, V, _want_trace=False):
    """Takes full inputs Q,K [8,8,1024,64], V [8,8,1024,1024]; returns [8,8,1024,1024]."""
    nc = _get_nc()
    Q = np.ascontiguousarray(np.asarray(Q), dtype=np.float32)
    K = np.ascontiguousarray(np.asarray(K), dtype=np.float32)
    V = np.ascontiguousarray(np.asarray(V), dtype=np.float32)
    in_maps = [
        {"q": Q[b], "k": K[b], "v": V[b]} for b in range(N_CORES)
    ]
    res = run_bass_kernel_spmd(
        nc, in_maps, core_ids=list(range(N_CORES)), trace=_want_trace
    )
    out = np.stack([res.results[b]["o"] for b in range(N_CORES)], axis=0)
    if _want_trace:
        _cache["last_result"] = res
    return out
